# revision 13
# baseline (speedup 1.0000x reference)
"""DeepLagrangianNetwork fused Bass/Tile kernel for 8 Trainium2 NeuronCores.

Strategy: pure data parallel over the batch (8192/8 = 1024 samples per core).
Per core, the batch is processed in tiles of 128 samples. For each tile the
MLP runs in feature-major layout ([feature_partition, col]) with an augmented
column space of 8*128 columns: block 0 holds the forward values y for the 128
samples, blocks 1..7 hold the forward-mode tangent values d y/d state_t.  The
two hidden 1024x1024 layers then become dense accumulated matmuls
([128K,128M] x [128K,512N]).  ReLU masking of the tangent blocks is a single
fused (y > 0) * der vector op per output chunk.  Head outputs (l_diag, l_low,
V') are transposed back to batch-major with the tensor engine and the per-
sample M/C/G/tau assembly is done with small strided/broadcast vector ops:

  w = L^T v, u = L^T a, dldt_e = sum_t der_l[e,t] v_t, wdot = Ddot^T v,
  C = L wdot + Ddot w - (P_t . w),  P_t[c] = sum_r dLdq_t[r,c] v_r,
  M = L L^T + eps I,  tau = L u + eps a + C + G.
"""

import os
import sys

for _p in (
    "/opt/trn_rl_repo",
    "/root/.axon_site/_ro/trn_rl_repo",
    "/root/.axon_site/_ro/pypackages",
):
    if os.path.isdir(_p) and _p not in sys.path:
        sys.path.append(_p)

import numpy as np

N_DOF = 7
WIDTH = 1024
BATCH = 8192
N_CORES = 8
B_CORE = BATCH // N_CORES
T = 128  # samples per tile
EPS = 1e-5
LSIZE = N_DOF * (N_DOF + 1) // 2  # 28
LOWER = LSIZE - N_DOF  # 21
NH = LSIZE + 1  # 29 head outputs: 7 diag, 21 lower, 1 V
KC = WIDTH // 128  # 8 feature chunks
AUGC = (1 + N_DOF) * T  # 1024 augmented columns per tile

# chunk offsets of the triangular index ordering: chunk i holds entries
# (r, c) = (i + j, j) for j in 0..6-i
TRI_OFF = [0, 7, 13, 18, 22, 25, 27]

# matmul dtype for the big accumulations: "float32" (exact, 4 cyc/row) or
# "float32r" (fast fp32 mode, 1 cyc/row at N>=256)
MM_DTYPE = os.environ.get("DELAN_MM_DTYPE", "float32r")

_CACHE = {}


def _build(b_core=B_CORE, mm_dtype=MM_DTYPE):
    key = (b_core, mm_dtype)
    if key in _CACHE:
        return _CACHE[key]

    import concourse.bacc as bacc
    import concourse.tile as tile
    import concourse.mybir as mybir
    from concourse.masks import make_identity

    dt = mybir.dt
    f32 = dt.float32
    mmdt = getattr(dt, mm_dtype)
    AF = mybir.ActivationFunctionType
    ALU = mybir.AluOpType
    AX = mybir.AxisListType

    n_tiles = b_core // T
    assert b_core % T == 0

    nc = bacc.Bacc("TRN2", target_bir_lowering=False, debug=False)

    def din(name, shape):
        return nc.dram_tensor(name, shape, f32, kind="ExternalInput").ap()

    def dout(name, shape):
        return nc.dram_tensor(name, shape, f32, kind="ExternalOutput").ap()

    state = din("state", [b_core, N_DOF])
    vel = din("velocity", [b_core, N_DOF])
    acc = din("acceleration", [b_core, N_DOF])
    W_in = din("W_in", [N_DOF, WIDTH])
    b_in = din("b_in", [WIDTH])
    W_h = din("W_h", [WIDTH, WIDTH])
    b_h = din("b_h", [WIDTH])
    W_g = din("W_g", [WIDTH, 1])
    b_g = din("b_g", [1])
    W_ld = din("W_ld", [WIDTH, N_DOF])
    b_ld = din("b_ld", [N_DOF])
    W_lo = din("W_lo", [WIDTH, LOWER])
    b_lo = din("b_lo", [LOWER])

    tau_out = dout("tau", [b_core, N_DOF])
    M_out = dout("M", [b_core, N_DOF * N_DOF])
    C_out = dout("C", [b_core, N_DOF])
    G_out = dout("G", [b_core, N_DOF])

    # tensors consumed by an fp32r matmul must be *written* as float32r
    # (the BIR verifier enforces pre-rounded producers), so the Y
    # activations and the hidden/head weights carry mmdt directly
    ydt = mmdt

    with tile.TileContext(nc) as tc:
        with (
            tc.tile_pool(name="consts", bufs=1) as consts,
            tc.tile_pool(name="ypool", bufs=3) as ypool,
            tc.tile_pool(name="hd", bufs=2) as hdp,
            tc.tile_pool(name="td", bufs=2) as tdp,
            tc.tile_pool(name="inp", bufs=2) as inp,
            tc.tile_pool(name="scr", bufs=2) as scr,
            tc.tile_pool(name="ps_a", bufs=3, space="PSUM") as ps_a,
            tc.tile_pool(name="ps_tp", bufs=2, space="PSUM") as ps_tp,
        ):
            # ---- constants -------------------------------------------------
            ident = consts.tile([128, 128], f32)
            make_identity(nc, ident[:])

            wh_sb = consts.tile([128, KC * WIDTH], ydt)  # [p, kc*1024 + n]
            if ydt is f32:
                nc.sync.dma_start(
                    wh_sb[:].rearrange("p (kc n) -> p kc n", kc=KC),
                    W_h.rearrange("(kc p) n -> p kc n", p=128),
                )
            else:
                wh_raw = consts.tile([128, KC * WIDTH], f32)
                nc.sync.dma_start(
                    wh_raw[:].rearrange("p (kc n) -> p kc n", kc=KC),
                    W_h.rearrange("(kc p) n -> p kc n", p=128),
                )
                nc.vector.tensor_copy(wh_sb[:], wh_raw[:])

            win_sb = consts.tile([N_DOF, WIDTH], f32)
            nc.sync.dma_start(win_sb[:], W_in)

            # W_in^T per output chunk: [p, oc*7 + t] = W_in[t, oc*128 + p]
            wint_sb = consts.tile([128, KC * N_DOF], f32)
            for oc in range(KC):
                nc.sync.dma_start(
                    wint_sb[:, oc * N_DOF : (oc + 1) * N_DOF],
                    W_in[:, oc * 128 : (oc + 1) * 128].transpose([1, 0]),
                )

            # concat head weights: [p, kc*29 + h]
            wcat_sb = consts.tile([128, KC * NH], ydt)
            wcat_raw = consts.tile([128, KC * NH], f32)
            wcat3 = wcat_raw[:].rearrange("p (kc h) -> p kc h", kc=KC)
            nc.sync.dma_start(
                wcat3[:, :, 0:N_DOF], W_ld.rearrange("(kc p) h -> p kc h", p=128)
            )
            nc.sync.dma_start(
                wcat3[:, :, N_DOF:LSIZE],
                W_lo.rearrange("(kc p) h -> p kc h", p=128),
            )
            nc.sync.dma_start(
                wcat3[:, :, LSIZE:NH], W_g.rearrange("(kc p) h -> p kc h", p=128)
            )
            nc.vector.tensor_copy(wcat_sb[:], wcat_raw[:])

            bin_sb = consts.tile([128, KC], f32)
            nc.sync.dma_start(bin_sb[:], b_in.rearrange("(oc p) -> p oc", p=128))
            bh_sb = consts.tile([128, KC], f32)
            nc.sync.dma_start(bh_sb[:], b_h.rearrange("(oc p) -> p oc", p=128))

            # concat head bias via a DRAM bounce (SBUF APs must start at a
            # partition multiple of 32, so build the concat in DRAM first)
            bcat_dram = nc.dram_tensor("bcat_scratch", [NH], f32).ap()
            nc.sync.dma_start(bcat_dram[0:N_DOF], b_ld)
            nc.sync.dma_start(bcat_dram[N_DOF:LSIZE], b_lo)
            nc.sync.dma_start(bcat_dram[LSIZE:NH], b_g)
            bcat_sb = consts.tile([NH, 1], f32)
            nc.sync.dma_start(bcat_sb[:], bcat_dram.unsqueeze(1))

            # augmented layer-1 rhs: block 0 gets state^T per tile; blocks
            # 1..7 are the constant tangent seed (rows of the identity)
            aug0 = consts.tile([N_DOF, AUGC], f32)
            nc.gpsimd.memset(aug0[:, T:AUGC], 0.0)
            # E[i, t*T + j] = (i == t): keep 0 where (i - t) != 0, else fill 1
            nc.gpsimd.affine_select(
                out=aug0[:, T:AUGC].rearrange("p (t b) -> p t b", t=N_DOF),
                in_=aug0[:, T:AUGC].rearrange("p (t b) -> p t b", t=N_DOF),
                compare_op=mybir.AluOpType.not_equal,
                fill=1.0,
                base=0,
                pattern=[[-1, N_DOF], [0, T]],
                channel_multiplier=1,
            )

            # ---- per-tile pipeline ----------------------------------------
            for it in range(n_tiles):
                rows = slice(it * T, (it + 1) * T)

                state_nat = inp.tile([T, N_DOF], f32, tag="state")
                nc.sync.dma_start(state_nat[:], state[rows, :])
                v_nat = inp.tile([T, N_DOF], f32, tag="vel")
                nc.sync.dma_start(v_nat[:], vel[rows, :])
                a_nat = inp.tile([T, N_DOF], f32, tag="acc")
                nc.sync.dma_start(a_nat[:], acc[rows, :])

                # state^T into aug0 block 0 (PE transpose via identity)
                ps_s = ps_tp.tile([128, 128], f32, tag="ps_small")
                nc.tensor.transpose(ps_s[0:N_DOF, 0:T], state_nat[:], ident[:])
                nc.vector.tensor_copy(aug0[:, 0:T], ps_s[0:N_DOF, 0:T])

                # ---- layer 1 (7 -> 1024) ----
                Y1 = ypool.tile([128, KC * AUGC], ydt, tag="Y")
                for oc in range(KC):
                    ps_y = ps_tp.tile([128, 128], f32, tag="ps_small")
                    nc.tensor.matmul(
                        ps_y[:, 0:T],
                        win_sb[:, oc * 128 : (oc + 1) * 128],
                        aug0[:, 0:T],
                        start=True,
                        stop=True,
                    )
                    yblk = Y1[:, oc * AUGC : oc * AUGC + T]
                    nc.scalar.activation(
                        yblk, ps_y[:, 0:T], AF.Relu, bias=bin_sb[:, oc : oc + 1]
                    )
                    # d1 = (a1 > 0) * W_in[t, o]
                    nc.vector.scalar_tensor_tensor(
                        out=Y1[:, oc * AUGC + T : (oc + 1) * AUGC].rearrange(
                            "p (t b) -> p t b", t=N_DOF
                        ),
                        in0=yblk.bitcast(f32).unsqueeze(1).broadcast_to([128, N_DOF, T]),
                        scalar=0.0,
                        in1=wint_sb[:, oc * N_DOF : (oc + 1) * N_DOF]
                        .unsqueeze(2)
                        .broadcast_to([128, N_DOF, T]),
                        op0=ALU.is_gt,
                        op1=ALU.mult,
                    )

                # ---- layers 2 and 3 (1024 -> 1024, shared W_h) ----
                Yprev = Y1
                for layer in (2, 3):
                    Ycur = ypool.tile([128, KC * AUGC], ydt, tag="Y")
                    for oc in range(KC):
                        ps = ps_a.tile([128, AUGC], f32, tag="ps_a")
                        for kc in range(KC):
                            lhsT = wh_sb[
                                :, kc * WIDTH + oc * 128 : kc * WIDTH + (oc + 1) * 128
                            ]
                            for h in range(2):
                                nc.tensor.matmul(
                                    ps[:, h * 512 : (h + 1) * 512],
                                    lhsT,
                                    Yprev[
                                        :,
                                        kc * AUGC + h * 512 : kc * AUGC + (h + 1) * 512,
                                    ],
                                    start=(kc == 0),
                                    stop=(kc == KC - 1),
                                )
                        yblk = Ycur[:, oc * AUGC : oc * AUGC + T]
                        nc.scalar.activation(
                            yblk, ps[:, 0:T], AF.Relu, bias=bh_sb[:, oc : oc + 1]
                        )
                        nc.vector.scalar_tensor_tensor(
                            out=Ycur[:, oc * AUGC + T : (oc + 1) * AUGC].rearrange(
                                "p (t b) -> p t b", t=N_DOF
                            ),
                            in0=yblk.bitcast(f32).unsqueeze(1).broadcast_to([128, N_DOF, T]),
                            scalar=0.0,
                            in1=ps[:, T:AUGC].rearrange("p (t b) -> p t b", t=N_DOF),
                            op0=ALU.is_gt,
                            op1=ALU.mult,
                        )
                    Yprev = Ycur

                # ---- heads (1024 -> 29) ----
                ps_h = ps_a.tile([128, AUGC], f32, tag="ps_a")
                for kc in range(KC):
                    lhsT = wcat_sb[:, kc * NH : (kc + 1) * NH]
                    for h in range(2):
                        nc.tensor.matmul(
                            ps_h[0:NH, h * 512 : (h + 1) * 512],
                            lhsT,
                            Yprev[:, kc * AUGC + h * 512 : kc * AUGC + (h + 1) * 512],
                            start=(kc == 0),
                            stop=(kc == KC - 1),
                        )
                hd = hdp.tile([NH, AUGC], f32, tag="hd")
                # forward values: pre-activation + bias for all 29 rows
                # (SBUF partition starts must be 0/32/64/96, so operate on
                # full row ranges and fix up the diag rows afterwards)
                nc.scalar.activation(
                    hd[:, 0:T], ps_h[0:NH, 0:T], AF.Identity, bias=bcat_sb[:]
                )
                # tangents: copy all rows, then overwrite diag rows masked
                nc.vector.tensor_copy(hd[:, T:AUGC], ps_h[0:NH, T:AUGC])
                nc.vector.scalar_tensor_tensor(
                    out=hd[0:N_DOF, T:AUGC].rearrange("p (t b) -> p t b", t=N_DOF),
                    in0=hd[0:N_DOF, 0:T].unsqueeze(1).broadcast_to([N_DOF, N_DOF, T]),
                    scalar=0.0,
                    in1=ps_h[0:N_DOF, T:AUGC].rearrange("p (t b) -> p t b", t=N_DOF),
                    op0=ALU.is_gt,
                    op1=ALU.mult,
                )
                # l_diag = relu(pre): in-place clamp AFTER the mask used pre>0
                nc.vector.tensor_scalar_max(hd[0:N_DOF, 0:T], hd[0:N_DOF, 0:T], 0.0)

                # ---- transpose to batch-major: td[s, blk*29 + e] ----
                td = tdp.tile([T, (1 + N_DOF) * NH], f32, tag="td")
                for blk in range(1 + N_DOF):
                    ps_t = ps_tp.tile([128, 128], f32, tag="ps_small")
                    nc.tensor.transpose(
                        ps_t[0:T, 0:NH],
                        hd[:, blk * T : (blk + 1) * T],
                        ident[0:NH, 0:NH],
                    )
                    nc.vector.tensor_copy(
                        td[:, blk * NH : (blk + 1) * NH], ps_t[0:T, 0:NH]
                    )

                # ---- per-sample assembly (batch-major) ----
                D = N_DOF  # 7
                der0 = NH  # start of tangent blocks in td

                def tri_expand(dst49, src28ap):
                    """scatter tri entries into a dense 7x7 [p, 7r+c] matrix"""
                    nc.gpsimd.memset(dst49[:], 0.0)
                    for i in range(D):
                        n = D - i
                        nc.vector.tensor_copy(
                            dst49[:, 7 * i : 7 * i + 8 * (n - 1) + 1 : 8],
                            src28ap[:, TRI_OFF[i] : TRI_OFF[i] + n],
                        )

                lmat = scr.tile([T, 49], f32, tag="lmat")
                tri_expand(lmat, td)

                # w = L^T v ; u = L^T a   (prod over (c outer later) r)
                t49a = scr.tile([T, 49], f32, tag="t49a")
                w7 = scr.tile([T, D], f32, tag="w7")
                u7 = scr.tile([T, D], f32, tag="u7")
                for vec, out7 in ((v_nat, w7), (a_nat, u7)):
                    nc.vector.tensor_tensor(
                        out=t49a[:].rearrange("p (r c) -> p r c", r=D),
                        in0=lmat[:].rearrange("p (r c) -> p r c", r=D),
                        in1=vec[:, 0:D].unsqueeze(2).broadcast_to([T, D, D]),
                        op=ALU.mult,
                    )
                    nc.vector.tensor_reduce(
                        out=out7[:],
                        in_=t49a[:].rearrange("p (r c) -> p c r", r=D),
                        axis=AX.X,
                        op=ALU.add,
                    )

                # dldt[e] = sum_t der[e, t] * v[t]
                der_et = (
                    td[:, der0 : der0 + D * NH]
                    .rearrange("p (t ee) -> p ee t", t=D)[:, 0:LSIZE, :]
                )
                prod196 = scr.tile([T, LSIZE * D], f32, tag="prod196")
                nc.vector.tensor_tensor(
                    out=prod196[:].rearrange("p (e t) -> p e t", e=LSIZE),
                    in0=der_et,
                    in1=v_nat[:, 0:D].unsqueeze(1).broadcast_to([T, LSIZE, D]),
                    op=ALU.mult,
                )
                dldt = scr.tile([T, LSIZE], f32, tag="dldt")
                nc.vector.tensor_reduce(
                    out=dldt[:],
                    in_=prod196[:].rearrange("p (e t) -> p e t", e=LSIZE),
                    axis=AX.X,
                    op=ALU.add,
                )

                dmat = scr.tile([T, 49], f32, tag="dmat")
                tri_expand(dmat, dldt)

                # wdot = Ddot^T v
                wd7 = scr.tile([T, D], f32, tag="wd7")
                nc.vector.tensor_tensor(
                    out=t49a[:].rearrange("p (r c) -> p r c", r=D),
                    in0=dmat[:].rearrange("p (r c) -> p r c", r=D),
                    in1=v_nat[:, 0:D].unsqueeze(2).broadcast_to([T, D, D]),
                    op=ALU.mult,
                )
                nc.vector.tensor_reduce(
                    out=wd7[:],
                    in_=t49a[:].rearrange("p (r c) -> p c r", r=D),
                    axis=AX.X,
                    op=ALU.add,
                )

                # row-wise products: X[r] = sum_c mat[r,c] * y[c]
                def matvec_rows(out7, mat49, y7):
                    nc.vector.tensor_tensor(
                        out=t49a[:].rearrange("p (r c) -> p r c", r=D),
                        in0=mat49[:].rearrange("p (r c) -> p r c", r=D),
                        in1=y7[:, 0:D].unsqueeze(1).broadcast_to([T, D, D]),
                        op=ALU.mult,
                    )
                    nc.vector.tensor_reduce(
                        out=out7[:],
                        in_=t49a[:].rearrange("p (r c) -> p r c", r=D),
                        axis=AX.X,
                        op=ALU.add,
                    )

                lw7 = scr.tile([T, D], f32, tag="lw7")
                matvec_rows(lw7, dmat, w7)  # Ddot w
                dw7 = scr.tile([T, D], f32, tag="dw7")
                matvec_rows(dw7, lmat, wd7)  # L wdot
                lu7 = scr.tile([T, D], f32, tag="lu7")
                matvec_rows(lu7, lmat, u7)  # L u

                # P[t, c] = sum over entries (r,c) of der[e, t] * v[r]
                p_all = scr.tile([T, D * D], f32, tag="p_all")
                tp7 = scr.tile([T, D * D], f32, tag="tp7")
                der_te = td[:, der0 : der0 + D * NH].rearrange(
                    "p (t ee) -> p t ee", t=D
                )
                for i in range(D):
                    n = D - i
                    if i == 0:
                        outv = p_all[:].rearrange("p (t c) -> p t c", t=D)
                    else:
                        outv = tp7[:, 0 : D * n].rearrange("p (t j) -> p t j", t=D)
                    nc.vector.tensor_tensor(
                        out=outv,
                        in0=der_te[:, :, TRI_OFF[i] : TRI_OFF[i] + n],
                        in1=v_nat[:, i:D].unsqueeze(1).broadcast_to([T, D, n]),
                        op=ALU.mult,
                    )
                    if i > 0:
                        pview = p_all[:].rearrange("p (t c) -> p t c", t=D)[:, :, 0:n]
                        nc.vector.tensor_tensor(
                            out=pview, in0=pview, in1=outv, op=ALU.add
                        )

                # pw[t] = P[t, :] . w
                pw7 = scr.tile([T, D], f32, tag="pw7")
                nc.vector.tensor_tensor(
                    out=t49a[:].rearrange("p (t c) -> p t c", t=D),
                    in0=p_all[:].rearrange("p (t c) -> p t c", t=D),
                    in1=w7[:, 0:D].unsqueeze(1).broadcast_to([T, D, D]),
                    op=ALU.mult,
                )
                nc.vector.tensor_reduce(
                    out=pw7[:],
                    in_=t49a[:].rearrange("p (t c) -> p t c", t=D),
                    axis=AX.X,
                    op=ALU.add,
                )

                # M = L L^T + eps I
                t343 = scr.tile([T, 343], f32, tag="t343")
                nc.vector.tensor_tensor(
                    out=t343[:].rearrange("p (r c k) -> p r c k", r=D, c=D),
                    in0=lmat[:]
                    .rearrange("p (r k) -> p r k", r=D)
                    .unsqueeze(2)
                    .broadcast_to([T, D, D, D]),
                    in1=lmat[:]
                    .rearrange("p (c k) -> p c k", c=D)
                    .unsqueeze(1)
                    .broadcast_to([T, D, D, D]),
                    op=ALU.mult,
                )
                m49 = scr.tile([T, 49], f32, tag="m49")
                nc.vector.tensor_reduce(
                    out=m49[:],
                    in_=t343[:].rearrange("p (rc k) -> p rc k", k=D),
                    axis=AX.X,
                    op=ALU.add,
                )
                nc.vector.tensor_scalar_add(m49[:, 0:49:8], m49[:, 0:49:8], EPS)

                # C = (L wdot + Ddot w) - P.w ; tau = L u + eps a + C + G
                c7 = scr.tile([T, D], f32, tag="c7")
                t7 = scr.tile([T, D], f32, tag="t7")
                nc.vector.tensor_add(t7[:], lw7[:], dw7[:])
                nc.vector.tensor_sub(c7[:], t7[:], pw7[:])

                md7 = scr.tile([T, D], f32, tag="md7")
                nc.vector.scalar_tensor_tensor(
                    out=md7[:],
                    in0=a_nat[:, 0:D],
                    scalar=EPS,
                    in1=lu7[:],
                    op0=ALU.mult,
                    op1=ALU.add,
                )
                tau7 = scr.tile([T, D], f32, tag="tau7")
                g_ap = td[:, der0 + LSIZE : der0 + LSIZE + (D - 1) * NH + 1 : NH]
                nc.vector.tensor_add(t7[:], md7[:], c7[:])
                nc.vector.tensor_add(tau7[:], t7[:], g_ap)

                nc.sync.dma_start(tau_out[rows, :], tau7[:])
                nc.sync.dma_start(C_out[rows, :], c7[:])
                nc.sync.dma_start(M_out[rows, :], m49[:])
                nc.sync.dma_start(G_out[rows, :], g_ap)

    nc.compile()
    _CACHE[key] = nc
    return nc


def kernel(**inputs):
    from concourse.bass_utils import run_bass_kernel_spmd

    nc = _build()

    weights = {
        k: np.ascontiguousarray(inputs[k], dtype=np.float32)
        for k in (
            "W_in",
            "b_in",
            "W_h",
            "b_h",
            "W_g",
            "b_g",
            "W_ld",
            "b_ld",
            "W_lo",
            "b_lo",
        )
    }
    in_maps = []
    for c in range(N_CORES):
        rows = slice(c * B_CORE, (c + 1) * B_CORE)
        m = dict(weights)
        m["state"] = np.ascontiguousarray(inputs["state"][rows], dtype=np.float32)
        m["velocity"] = np.ascontiguousarray(
            inputs["velocity"][rows], dtype=np.float32
        )
        m["acceleration"] = np.ascontiguousarray(
            inputs["acceleration"][rows], dtype=np.float32
        )
        in_maps.append(m)

    res = run_bass_kernel_spmd(nc, in_maps, list(range(N_CORES))).results

    tau = np.concatenate([res[c]["tau"] for c in range(N_CORES)], axis=0)
    M = np.concatenate([res[c]["M"] for c in range(N_CORES)], axis=0).reshape(
        BATCH, N_DOF, N_DOF
    )
    C = np.concatenate([res[c]["C"] for c in range(N_CORES)], axis=0)
    G = np.concatenate([res[c]["G"] for c in range(N_CORES)], axis=0)
    return tau, M, C, G
